# revision 4
# baseline (speedup 1.0000x reference)
"""Differential attention (DiffAttn) kernel for 8 TRN2 NeuronCores.

Problem: B=4, T=4096, C=1024, one differential head (2x64 qk dims, 128 v dims),
causal, weights = softmax(q1k1/8) - lam * softmax(q2k2/8), out = weights @ v.

Sharding: pure data-parallel, zero collectives. 8 cores = 4 batches x 2
query-halves. The query rows are zigzag-interleaved at 256-row granularity
(core half h owns rows [512k + 256h, 512k + 256h + 256) for k=0..7) so both
halves have identical causal tile structure (SPMD: one graph for all cores)
and identical FLOPs.

Per-core pipeline (bf16 compute, fp32 accumulation):
  - host pre-transposes x (free): xT [C,T] so the C-contraction needs no
    on-chip transposes; host also gathers the core's own query columns (xq),
    pre-scales Wq by 1/8, computes lam, and builds the causal mask constants.
  - projections on PE: kT[128f, T], qT[128f, 2048] (feature-major, which is
    exactly the scores operand layout) and v[s, 128] (via vT + DMA-transpose).
  - scores in [t, s] layout: both 64-dim heads row-packed in one PE pass
    (tile_position (0,0)/(64,0)), accumulating 2x512-chunk groups in PSUM.
  - causal mask: host-built additive -30000 mask on the diagonal 512-chunk.
  - exp on ACT with accum_out row sums (no max-subtraction needed: scores
    are ~N(0,1) so exp never overflows; softmax is shift-invariant).
  - combine in ONE fused DVE op: p_neg = p2 * (lam*sum1/sum2) - p1
    (per-partition scalar), then DMA-transpose (xbar) the combined strip,
    PV matmul, and a final fused scale by -1/sum1 on eviction.
"""
import math
import os
import sys
import types
from contextlib import ExitStack

import ml_dtypes
import numpy as np


def _install_ntff_hook():
    """Make `antenv.axon_hooks` importable (the agent image ships a stub
    antenv without it), wiring the NTFF profile hook straight to the axon
    .so so run_bass_kernel_spmd(trace=True) can report HW exec time."""
    try:
        import antenv.axon_hooks  # noqa: F401
        return
    except Exception:
        pass
    try:
        import antenv
    except Exception:
        return
    mod = types.ModuleType("antenv.axon_hooks")
    mod._hook = None

    def set_axon_ntff_profile_hook(h):
        mod._hook = h

    def get_axon_ntff_profile_hook():
        if mod._hook is None:
            try:
                from trn_agent_boot.trn_boot import _ntff_profile_via_ctypes
                mod._hook = _ntff_profile_via_ctypes("/opt/axon/libaxon_pjrt.so")
            except Exception:
                mod._hook = None
        return mod._hook

    mod.set_axon_ntff_profile_hook = set_axon_ntff_profile_hook
    mod.get_axon_ntff_profile_hook = get_axon_ntff_profile_hook
    sys.modules["antenv.axon_hooks"] = mod
    antenv.axon_hooks = mod


_install_ntff_hook()

import concourse.bacc as bacc
import concourse.bass as bass
import concourse.bass_utils as _bass_utils
import concourse.tile as tile
from concourse import mybir
from concourse.bass_utils import run_bass_kernel_spmd

# zero-egress container: don't try to copy NEFF/NTFF artifacts to a bucket
_bass_utils.upload_artifacts = lambda tmpdir: f"local://{tmpdir}"

BF16 = mybir.dt.bfloat16
F32 = mybir.dt.float32
NPBF16 = ml_dtypes.bfloat16
ts = bass.ts

B, T, C = 4, 4096, 1024
HS, H2 = 64, 128
NSUB = 16          # 128-row query subtiles per core
ROWS = NSUB * 128  # 2048 query rows per core
MASK_NEG = -30000.0

LAST_EXEC_NS = None
_NC_CACHE = {}


def _t0(j, half):
    """Global first query row of subtile j on core-half `half`."""
    return 512 * (j // 2) + 128 * (j % 2) + 256 * half


def _build(lam: float):
    nc = bacc.Bacc()
    xT_e = nc.declare_dram_parameter("xT", [C, T], BF16, isOutput=False)
    xq_e = nc.declare_dram_parameter("xq", [C, ROWS], BF16, isOutput=False)
    wq_e = nc.declare_dram_parameter("wq", [C, H2], BF16, isOutput=False)
    wk_e = nc.declare_dram_parameter("wk", [C, H2], BF16, isOutput=False)
    wv_e = nc.declare_dram_parameter("wv", [C, H2], BF16, isOutput=False)
    cm_e = nc.declare_dram_parameter("cmask", [2, 128, 512], BF16, isOutput=False)
    out_e = nc.declare_dram_parameter("out", [NSUB, 128, H2], BF16, isOutput=True)

    Exp = mybir.ActivationFunctionType.Exp
    mult = mybir.AluOpType.mult
    sub = mybir.AluOpType.subtract
    add = mybir.AluOpType.add

    with ExitStack() as ctx:
        tc = ctx.enter_context(tile.TileContext(nc))
        const = ctx.enter_context(tc.tile_pool(name="const", bufs=1))
        xs_pool = ctx.enter_context(tc.tile_pool(name="xs", bufs=4))
        persist = ctx.enter_context(tc.tile_pool(name="persist", bufs=1))
        vt_pool = ctx.enter_context(tc.tile_pool(name="vt", bufs=2))
        p_pool = ctx.enter_context(tc.tile_pool(name="p", bufs=2))
        pt_pool = ctx.enter_context(tc.tile_pool(name="pt", bufs=2))
        small = ctx.enter_context(tc.tile_pool(name="small", bufs=4))
        osb_pool = ctx.enter_context(tc.tile_pool(name="osb", bufs=2))
        proj_ps = ctx.enter_context(tc.tile_pool(name="proj_ps", bufs=3, space="PSUM"))
        sc_ps = ctx.enter_context(tc.tile_pool(name="sc_ps", bufs=1, space="PSUM"))
        pv_ps = ctx.enter_context(tc.tile_pool(name="pv_ps", bufs=1, space="PSUM"))

        # --- constants ---
        wq_sb = const.tile([128, 8, 128], BF16)
        wk_sb = const.tile([128, 8, 128], BF16)
        wv_sb = const.tile([128, 8, 128], BF16)
        for c in range(8):
            nc.gpsimd.dma_start(wq_sb[:, c, :], wq_e[ts(c, 128), :])
            nc.gpsimd.dma_start(wk_sb[:, c, :], wk_e[ts(c, 128), :])
            nc.gpsimd.dma_start(wv_sb[:, c, :], wv_e[ts(c, 128), :])
        cm_sb = const.tile([128, 2, 512], BF16)
        for m in range(2):
            nc.gpsimd.dma_start(cm_sb[:, m, :], cm_e[m, :, :])

        # --- persistent projection outputs ---
        qT = persist.tile([128, ROWS], BF16)     # [q-feature, own t]
        kT = persist.tile([128, T], BF16)        # [k-feature, s]
        v_sb = persist.tile([128, 32, 128], BF16)  # [s%128, s//128, v-feature]

        def q_block(tb):
            ps = proj_ps.tile([128, 512], F32, tag="pp")
            for c in range(8):
                xs = xs_pool.tile([128, 512], BF16, tag="xs")
                nc.gpsimd.dma_start(xs[:], xq_e[ts(c, 128), ts(tb, 512)])
                nc.tensor.matmul(ps[:], wq_sb[:, c, :], xs[:],
                                 start=(c == 0), stop=(c == 7))
            nc.vector.tensor_copy(qT[:, ts(tb, 512)], ps[:])

        def kv_block(sb):
            psk = proj_ps.tile([128, 512], F32, tag="pp")
            psv = proj_ps.tile([128, 512], F32, tag="pp")
            for c in range(8):
                xs = xs_pool.tile([128, 512], BF16, tag="xs")
                nc.gpsimd.dma_start(xs[:], xT_e[ts(c, 128), ts(sb, 512)])
                nc.tensor.matmul(psk[:], wk_sb[:, c, :], xs[:],
                                 start=(c == 0), stop=(c == 7))
                nc.tensor.matmul(psv[:], wv_sb[:, c, :], xs[:],
                                 start=(c == 0), stop=(c == 7))
            nc.vector.tensor_copy(kT[:, ts(sb, 512)], psk[:])
            vt = vt_pool.tile([128, 512], BF16)
            nc.vector.tensor_copy(vt[:], psv[:])
            nc.sync.dma_start_transpose(v_sb[:, 4 * sb:4 * sb + 4, :], vt[:])

        def attention(j):
            nch = j // 2 + 1          # 512-wide key chunks covered
            ngr = (nch + 1) // 2      # 1024-wide exp groups per head
            p1 = p_pool.tile([128, nch, 512], BF16, tag="p1")
            p2 = p_pool.tile([128, nch, 512], BF16, tag="p2")
            sp1 = small.tile([128, 4], F32, tag="sp1")
            sp2 = small.tile([128, 4], F32, tag="sp2")
            for gi in range(ngr):
                used = min(2, nch - 2 * gi)
                for h, (p, sp) in ((0, (p1, sp1)), (1, (p2, sp2))):
                    sc = sc_ps.tile([128, 2, 512], F32, tag=f"sc{h}")
                    for qd in range(used):
                        ch = 2 * gi + qd
                        nc.tensor.matmul(
                            sc[:, qd, :],
                            qT[64 * h:64 * h + 64, ts(j, 128)],
                            kT[64 * h:64 * h + 64, ts(ch, 512)],
                            start=True, stop=True,
                            tile_position=(64 * h, 0))
                    if 2 * gi + used == nch:  # strip's diagonal chunk
                        nc.vector.tensor_add(sc[:, used - 1, :],
                                             sc[:, used - 1, :],
                                             cm_sb[:, j % 2, :])
                    nc.scalar.activation(p[:, 2 * gi:2 * gi + used, :],
                                         sc[:, 0:used, :], Exp,
                                         accum_out=sp[:, gi:gi + 1])
            sum1 = small.tile([128, 1], F32, tag="sum1")
            sum2 = small.tile([128, 1], F32, tag="sum2")
            nc.vector.tensor_reduce(sum1[:], sp1[:, 0:ngr],
                                    axis=mybir.AxisListType.X, op=add)
            nc.vector.tensor_reduce(sum2[:], sp2[:, 0:ngr],
                                    axis=mybir.AxisListType.X, op=add)
            r2 = small.tile([128, 1], F32, tag="r2")
            r1 = small.tile([128, 1], F32, tag="r1")
            gsc = small.tile([128, 1], F32, tag="gsc")
            nc.vector.reciprocal(r2[:], sum2[:])
            nc.vector.reciprocal(r1[:], sum1[:])
            # gsc = lam * sum1 / sum2
            nc.vector.scalar_tensor_tensor(gsc[:], sum1[:], float(lam), r2[:],
                                           op0=mult, op1=mult)
            # p_neg = p2 * gsc - p1   (one fused DVE pass over the strip)
            pn = p_pool.tile([128, nch, 512], BF16, tag="pn")
            nc.vector.scalar_tensor_tensor(pn[:], p2[:, 0:nch, :], gsc[:],
                                           p1[:, 0:nch, :], op0=mult, op1=sub)
            # transpose the whole combined strip in one xbar DMA
            pt = pt_pool.tile([128, 4 * nch, 128], BF16)
            nc.sync.dma_start_transpose(pt[:], pn[:])
            pv = pv_ps.tile([128, 128], F32)
            for cc in range(4 * nch):
                nc.tensor.matmul(pv[:], pt[:, cc, :], v_sb[:, cc, :],
                                 start=(cc == 0), stop=(cc == 4 * nch - 1))
            osb = osb_pool.tile([128, 128], BF16)
            # out = pv * r1 * (-1)  (fused negate undoes the p_neg sign flip)
            nc.vector.tensor_scalar(osb[:], pv[:], r1[:], -1.0,
                                    op0=mult, op1=mult)
            nc.sync.dma_start(out_e[j, :, :], osb[:])

        for sb in range(8):
            if sb < 4:
                q_block(sb)
            kv_block(sb)
            attention(2 * sb)
            attention(2 * sb + 1)

    nc.compile()
    return nc


def _lambda_init(depth):
    return 0.8 - 0.6 * math.exp(-0.3 * (depth + 1))


def kernel(x, Wq, Wk, Wv, lambda_q1, lambda_q2, lambda_k1, lambda_k2):
    global LAST_EXEC_NS
    x = np.asarray(x, dtype=np.float32)
    Wq = np.asarray(Wq, dtype=np.float32)
    Wk = np.asarray(Wk, dtype=np.float32)
    Wv = np.asarray(Wv, dtype=np.float32)
    lq1 = np.asarray(lambda_q1, dtype=np.float64)
    lq2 = np.asarray(lambda_q2, dtype=np.float64)
    lk1 = np.asarray(lambda_k1, dtype=np.float64)
    lk2 = np.asarray(lambda_k2, dtype=np.float64)

    lam = float(np.exp(np.dot(lq1, lk1)) - np.exp(np.dot(lq2, lk2))
                + _lambda_init(0))

    key = round(lam, 9)
    if key not in _NC_CACHE:
        _NC_CACHE[key] = _build(lam)
    nc = _NC_CACHE[key]

    wq_h = np.ascontiguousarray((Wq * 0.125).astype(NPBF16))
    wk_h = np.ascontiguousarray(Wk.astype(NPBF16))
    wv_h = np.ascontiguousarray(Wv.astype(NPBF16))

    xT = [np.ascontiguousarray(x[b].T.astype(NPBF16)) for b in range(B)]

    i_idx = np.arange(128)[:, None]
    j_idx = np.arange(512)[None, :]
    in_maps = []
    for core in range(8):
        b, half = core // 2, core % 2
        xq = np.ascontiguousarray(np.concatenate(
            [xT[b][:, _t0(j, half):_t0(j, half) + 128] for j in range(NSUB)],
            axis=1))
        cm = np.empty((2, 128, 512), dtype=NPBF16)
        for m in range(2):
            r = 128 * m + 256 * half
            cm[m] = np.where(i_idx + r >= j_idx, 0.0, MASK_NEG).astype(NPBF16)
        in_maps.append({"xT": xT[b], "xq": xq, "wq": wq_h, "wk": wk_h,
                        "wv": wv_h, "cmask": cm})

    try:
        res = run_bass_kernel_spmd(nc, in_maps, list(range(8)))
    except Exception:
        if os.environ.get("BASS_TRACE"):
            # profiling path failed; rerun untraced
            os.environ["BASS_NEVER_TRACE"] = "1"
            res = run_bass_kernel_spmd(nc, in_maps, list(range(8)))
        else:
            raise
    LAST_EXEC_NS = res.exec_time_ns

    out = np.empty((B, T, H2), dtype=np.float32)
    for core in range(8):
        b, half = core // 2, core % 2
        o = np.asarray(res.results[core]["out"]).astype(np.float32)
        for j in range(NSUB):
            t0 = _t0(j, half)
            out[b, t0:t0 + 128, :] = o[j]
    return out


# revision 8
# speedup vs baseline: 1.3987x; 1.3987x over previous
"""Differential attention (DiffAttn) kernel for 8 TRN2 NeuronCores.

Problem: B=4, T=4096, C=1024, one differential head (2x64 qk dims, 128 v dims),
causal, weights = softmax(q1k1/8) - lam * softmax(q2k2/8), out = weights @ v.

Sharding: pure data-parallel, zero collectives. 8 cores = 4 batches x 2
query-halves. The query rows are zigzag-interleaved at 256-row granularity
(core half h owns rows [512k + 256h, 512k + 256h + 256) for k=0..7) so both
halves have identical causal tile structure (SPMD: one graph for all cores)
and identical FLOPs.

Per-core pipeline (bf16 compute, fp32 accumulation):
  - host pre-transposes x (free): xT [C,T] so the C-contraction needs no
    on-chip transposes; host also gathers the core's own query columns (xq),
    pre-scales Wq by 1/8, computes lam, and builds the causal mask constants.
  - projections on PE: kT[128f, T], qT[128f, 2048] (feature-major, which is
    exactly the scores operand layout) and v[s, 128] (via vT + DMA-transpose).
  - scores in [t, s] layout: both 64-dim heads row-packed in one PE pass
    (tile_position (0,0)/(64,0)), accumulating 2x512-chunk groups in PSUM.
  - causal mask: host-built additive -30000 mask on the diagonal 512-chunk.
  - exp on ACT with accum_out row sums (no max-subtraction needed: scores
    are ~N(0,1) so exp never overflows; softmax is shift-invariant).
  - combine in ONE fused DVE op: p_neg = p2 * (lam*sum1/sum2) - p1
    (per-partition scalar), then DMA-transpose (xbar) the combined strip,
    PV matmul, and a final fused scale by -1/sum1 on eviction.
"""
import math
import os
import sys
import types
from contextlib import ExitStack

import ml_dtypes
import numpy as np


def _install_ntff_hook():
    """Make `antenv.axon_hooks` importable (the agent image ships a stub
    antenv without it), wiring the NTFF profile hook straight to the axon
    .so so run_bass_kernel_spmd(trace=True) can report HW exec time."""
    try:
        import antenv.axon_hooks  # noqa: F401
        return
    except Exception:
        pass
    try:
        import antenv
    except Exception:
        return
    mod = types.ModuleType("antenv.axon_hooks")
    mod._hook = None

    def set_axon_ntff_profile_hook(h):
        mod._hook = h

    def get_axon_ntff_profile_hook():
        if mod._hook is None:
            try:
                from trn_agent_boot.trn_boot import _ntff_profile_via_ctypes
                mod._hook = _ntff_profile_via_ctypes("/opt/axon/libaxon_pjrt.so")
            except Exception:
                mod._hook = None
        return mod._hook

    mod.set_axon_ntff_profile_hook = set_axon_ntff_profile_hook
    mod.get_axon_ntff_profile_hook = get_axon_ntff_profile_hook
    sys.modules["antenv.axon_hooks"] = mod
    antenv.axon_hooks = mod


_install_ntff_hook()

import concourse.bacc as bacc
import concourse.bass as bass
import concourse.bass_utils as _bass_utils
import concourse.tile as tile
from concourse import mybir
from concourse.bass_utils import run_bass_kernel_spmd

# zero-egress container: don't try to copy NEFF/NTFF artifacts to a bucket
_bass_utils.upload_artifacts = lambda tmpdir: f"local://{tmpdir}"

BF16 = mybir.dt.bfloat16
F32 = mybir.dt.float32
NPBF16 = ml_dtypes.bfloat16
ts = bass.ts

B, T, C = 4, 4096, 1024
HS, H2 = 64, 128
NSUB = 16          # 128-row query subtiles per core
ROWS = NSUB * 128  # 2048 query rows per core
MASK_NEG = -30000.0

LAST_EXEC_NS = None
_NC_CACHE = {}


def _t0(j, half):
    """Global first query row of subtile j on core-half `half`."""
    return 512 * (j // 2) + 128 * (j % 2) + 256 * half


def _build(lam: float):
    nc = bacc.Bacc()
    xT_e = nc.declare_dram_parameter("xT", [C, T], BF16, isOutput=False)
    xq_e = nc.declare_dram_parameter("xq", [C, ROWS], BF16, isOutput=False)
    wq_e = nc.declare_dram_parameter("wq", [C, H2], BF16, isOutput=False)
    wk_e = nc.declare_dram_parameter("wk", [C, H2], BF16, isOutput=False)
    wv_e = nc.declare_dram_parameter("wv", [C, H2], BF16, isOutput=False)
    cm_e = nc.declare_dram_parameter("cmask", [2, 128, 512], BF16, isOutput=False)
    out_e = nc.declare_dram_parameter("out", [NSUB, 128, H2], BF16, isOutput=True)

    Exp = mybir.ActivationFunctionType.Exp
    mult = mybir.AluOpType.mult
    sub = mybir.AluOpType.subtract
    add = mybir.AluOpType.add

    with ExitStack() as ctx:
        tc = ctx.enter_context(tile.TileContext(nc))
        const = ctx.enter_context(tc.tile_pool(name="const", bufs=1))
        xs_pool = ctx.enter_context(tc.tile_pool(name="xs", bufs=4))
        persist = ctx.enter_context(tc.tile_pool(name="persist", bufs=1))
        vt_pool = ctx.enter_context(tc.tile_pool(name="vt", bufs=2))
        p_pool = ctx.enter_context(tc.tile_pool(name="p", bufs=2))
        pt_pool = ctx.enter_context(tc.tile_pool(name="pt", bufs=2))
        small = ctx.enter_context(tc.tile_pool(name="small", bufs=4))
        osb_pool = ctx.enter_context(tc.tile_pool(name="osb", bufs=2))
        proj_ps = ctx.enter_context(tc.tile_pool(name="proj_ps", bufs=3, space="PSUM"))
        sc_ps = ctx.enter_context(tc.tile_pool(name="sc_ps", bufs=1, space="PSUM"))
        pv_ps = ctx.enter_context(tc.tile_pool(name="pv_ps", bufs=1, space="PSUM"))

        # --- constants ---
        wq_sb = const.tile([128, 8, 128], BF16)
        wk_sb = const.tile([128, 8, 128], BF16)
        wv_sb = const.tile([128, 8, 128], BF16)
        for c in range(8):
            nc.gpsimd.dma_start(wq_sb[:, c, :], wq_e[ts(c, 128), :])
            nc.gpsimd.dma_start(wk_sb[:, c, :], wk_e[ts(c, 128), :])
            nc.gpsimd.dma_start(wv_sb[:, c, :], wv_e[ts(c, 128), :])
        cm_sb = const.tile([128, 2, 512], BF16)
        for m in range(2):
            nc.gpsimd.dma_start(cm_sb[:, m, :], cm_e[m, :, :])

        # --- resident x^T: 8 big DMAs (8KB contiguous rows) ---
        xt_sb = const.tile([128, 8, T], BF16)
        for c in range(8):
            nc.gpsimd.dma_start(xt_sb[:, c, :], xT_e[ts(c, 128), :])

        # --- persistent projection outputs ---
        qT = persist.tile([128, ROWS], BF16)     # [q-feature, own t]
        kT = persist.tile([128, T], BF16)        # [k-feature, s]
        v_sb = persist.tile([128, 32, 128], BF16)  # [s%128, s//128, v-feature]

        def q_block(tb):
            ps = proj_ps.tile([128, 512], F32, tag="pp")
            for c in range(8):
                xs = xs_pool.tile([128, 512], BF16, tag="xs")
                nc.scalar.dma_start(xs[:], xq_e[ts(c, 128), ts(tb, 512)])
                nc.tensor.matmul(ps[:], wq_sb[:, c, :], xs[:],
                                 start=(c == 0), stop=(c == 7))
            nc.vector.tensor_copy(qT[:, ts(tb, 512)], ps[:])

        def kv_block(sb):
            psk = proj_ps.tile([128, 512], F32, tag="pp")
            psv = proj_ps.tile([128, 512], F32, tag="pp")
            for c in range(8):
                nc.tensor.matmul(psk[:], wk_sb[:, c, :], xt_sb[:, c, ts(sb, 512)],
                                 start=(c == 0), stop=(c == 7))
                nc.tensor.matmul(psv[:], wv_sb[:, c, :], xt_sb[:, c, ts(sb, 512)],
                                 start=(c == 0), stop=(c == 7))
            nc.vector.tensor_copy(kT[:, ts(sb, 512)], psk[:])
            vt = vt_pool.tile([128, 512], BF16)
            nc.vector.tensor_copy(vt[:], psv[:])
            nc.sync.dma_start_transpose(v_sb[:, 4 * sb:4 * sb + 4, :], vt[:])

        def attention(j):
            nch = j // 2 + 1          # 512-wide key chunks covered
            ngr = (nch + 1) // 2      # 1024-wide exp groups per head
            p1 = p_pool.tile([128, nch, 512], BF16, tag="p1")
            p2 = p_pool.tile([128, nch, 512], BF16, tag="p2")
            sp1 = small.tile([128, 4], F32, tag="sp1")
            sp2 = small.tile([128, 4], F32, tag="sp2")
            for gi in range(ngr):
                used = min(2, nch - 2 * gi)
                for h, (p, sp) in ((0, (p1, sp1)), (1, (p2, sp2))):
                    sc = sc_ps.tile([128, 2, 512], F32, tag=f"sc{h}")
                    for qd in range(used):
                        ch = 2 * gi + qd
                        nc.tensor.matmul(
                            sc[:, qd, :],
                            qT[64 * h:64 * h + 64, ts(j, 128)],
                            kT[64 * h:64 * h + 64, ts(ch, 512)],
                            start=True, stop=True)
                    if 2 * gi + used == nch:  # strip's diagonal chunk
                        nc.vector.tensor_add(sc[:, used - 1, :],
                                             sc[:, used - 1, :],
                                             cm_sb[:, j % 2, :])
                    nc.scalar.activation(p[:, 2 * gi:2 * gi + used, :],
                                         sc[:, 0:used, :], Exp,
                                         accum_out=sp[:, gi:gi + 1])
            sum1 = small.tile([128, 1], F32, tag="sum1")
            sum2 = small.tile([128, 1], F32, tag="sum2")
            nc.vector.tensor_reduce(sum1[:], sp1[:, 0:ngr],
                                    axis=mybir.AxisListType.X, op=add)
            nc.vector.tensor_reduce(sum2[:], sp2[:, 0:ngr],
                                    axis=mybir.AxisListType.X, op=add)
            r2 = small.tile([128, 1], F32, tag="r2")
            r1 = small.tile([128, 1], F32, tag="r1")
            gsc = small.tile([128, 1], F32, tag="gsc")
            nc.vector.reciprocal(r2[:], sum2[:])
            nc.vector.reciprocal(r1[:], sum1[:])
            # gsc = lam * sum1 / sum2
            nc.vector.scalar_tensor_tensor(gsc[:], sum1[:], float(lam), r2[:],
                                           op0=mult, op1=mult)
            # p_neg = p2 * gsc - p1   (one fused DVE pass over the strip)
            pn = p_pool.tile([128, nch, 512], BF16, tag="pn")
            nc.vector.scalar_tensor_tensor(pn[:], p2[:, 0:nch, :], gsc[:],
                                           p1[:, 0:nch, :], op0=mult, op1=sub)
            # transpose the whole combined strip in one xbar DMA
            pt = pt_pool.tile([128, 4 * nch, 128], BF16)
            nc.sync.dma_start_transpose(pt[:], pn[:])
            pv = pv_ps.tile([128, 128], F32)
            for cc in range(4 * nch):
                nc.tensor.matmul(pv[:], pt[:, cc, :], v_sb[:, cc, :],
                                 start=(cc == 0), stop=(cc == 4 * nch - 1))
            osb = osb_pool.tile([128, 128], BF16)
            # out = pv * r1 * (-1)  (fused negate undoes the p_neg sign flip)
            nc.vector.tensor_scalar(osb[:], pv[:], r1[:], -1.0,
                                    op0=mult, op1=mult)
            nc.gpsimd.dma_start(out_e[j, :, :], osb[:])

        for sb in range(8):
            if sb < 4:
                q_block(sb)
            kv_block(sb)
            attention(2 * sb)
            attention(2 * sb + 1)

    nc.compile()
    return nc


def _lambda_init(depth):
    return 0.8 - 0.6 * math.exp(-0.3 * (depth + 1))


def kernel(x, Wq, Wk, Wv, lambda_q1, lambda_q2, lambda_k1, lambda_k2):
    global LAST_EXEC_NS
    x = np.asarray(x, dtype=np.float32)
    Wq = np.asarray(Wq, dtype=np.float32)
    Wk = np.asarray(Wk, dtype=np.float32)
    Wv = np.asarray(Wv, dtype=np.float32)
    lq1 = np.asarray(lambda_q1, dtype=np.float64)
    lq2 = np.asarray(lambda_q2, dtype=np.float64)
    lk1 = np.asarray(lambda_k1, dtype=np.float64)
    lk2 = np.asarray(lambda_k2, dtype=np.float64)

    lam = float(np.exp(np.dot(lq1, lk1)) - np.exp(np.dot(lq2, lk2))
                + _lambda_init(0))

    key = round(lam, 9)
    if key not in _NC_CACHE:
        _NC_CACHE[key] = _build(lam)
    nc = _NC_CACHE[key]

    wq_h = np.ascontiguousarray((Wq * 0.125).astype(NPBF16))
    wk_h = np.ascontiguousarray(Wk.astype(NPBF16))
    wv_h = np.ascontiguousarray(Wv.astype(NPBF16))

    xT = [np.ascontiguousarray(x[b].T.astype(NPBF16)) for b in range(B)]

    i_idx = np.arange(128)[:, None]
    j_idx = np.arange(512)[None, :]
    in_maps = []
    for core in range(8):
        b, half = core // 2, core % 2
        xq = np.ascontiguousarray(np.concatenate(
            [xT[b][:, _t0(j, half):_t0(j, half) + 128] for j in range(NSUB)],
            axis=1))
        cm = np.empty((2, 128, 512), dtype=NPBF16)
        for m in range(2):
            r = 128 * m + 256 * half
            cm[m] = np.where(i_idx + r >= j_idx, 0.0, MASK_NEG).astype(NPBF16)
        in_maps.append({"xT": xT[b], "xq": xq, "wq": wq_h, "wk": wk_h,
                        "wv": wv_h, "cmask": cm})

    try:
        res = run_bass_kernel_spmd(nc, in_maps, list(range(8)))
    except Exception:
        if os.environ.get("BASS_TRACE"):
            # profiling path failed; rerun untraced
            os.environ["BASS_NEVER_TRACE"] = "1"
            res = run_bass_kernel_spmd(nc, in_maps, list(range(8)))
        else:
            raise
    LAST_EXEC_NS = res.exec_time_ns

    out = np.empty((B, T, H2), dtype=np.float32)
    for core in range(8):
        b, half = core // 2, core % 2
        o = np.asarray(res.results[core]["out"]).astype(np.float32)
        for j in range(NSUB):
            t0 = _t0(j, half)
            out[b, t0:t0 + 128, :] = o[j]
    return out
